# revision 17
# baseline (speedup 1.0000x reference)
"""Trainium2 Bass kernel for nn_BatteryMoEFlattenIntraCycleMoELayer.

Computes, for B=128 battery samples of shape [L=128, F=900]:
    g = renorm(top_k(softmax(logits) * mask))          # [B, E=8] gates
    out[b] = sum_e g[b,e] * (x[b] @ W[e] + b[e])       # [B, L, D=512], bf16

Strategy: data-parallel over B across 8 cores (16 samples/core), W replicated.
Host does gating (tiny [128,8] math) + layout/dtype glue only; all O(B*L*F*D)
FLOPs run on device.  Per sample the device computes the two selected experts'
matmuls in bf16 (x pre-scaled by the gate), accumulating both experts and the
gated bias (via a ones-row folded into the K dimension) into one PSUM bank.
Expert selection is a runtime register read (values_load) driving a dynamic
access pattern into SBUF-resident W.
"""

import os

import numpy as np
import ml_dtypes

B, L, FIN, D, E = 128, 128, 900, 512, 8
NCORES = 8
BPC = B // NCORES            # samples per core
NCH = FIN // 128             # 7 full 128-row contraction chunks
TAILF = FIN - NCH * 128      # 4 leftover features
TAIL = TAILF + 1             # + ones row (folds bias into the matmul)
EPS = 1e-9

BF16 = ml_dtypes.bfloat16

LAST_RESULT = None           # BassKernelResults of the most recent run
_PROG_CACHE = {}


def _gates_np(logits, moe_masks, top_k):
    """Numpy replica of reference._gates (f32, same tie-breaking as lax.top_k)."""
    logits = logits.astype(np.float32)
    mask = (moe_masks == 1).astype(np.float32)
    x = logits - logits.max(axis=1, keepdims=True)
    ex = np.exp(x)
    sm = ex / ex.sum(axis=1, keepdims=True)
    g = sm * mask
    if top_k > 0:
        idx = np.argsort(-g, axis=1, kind="stable")[:, :top_k]
        tk = np.zeros_like(g)
        np.put_along_axis(tk, idx, np.float32(1.0), axis=1)
        g = g * tk
    return g / (g.sum(axis=1, keepdims=True) + np.float32(EPS))


def _build_program(nslot, group):
    """Build the (SPMD, per-core) Bass program: BPC samples, nslot expert
    slots per sample, `group` samples sharing a round of PSUM banks."""
    import concourse.mybir as mybir
    import concourse.tile as tile
    from concourse import bacc
    from concourse.bass import ds

    bf = mybir.dt.bfloat16
    f32 = mybir.dt.float32
    i32 = mybir.dt.int32

    nc = bacc.Bacc("TRN2", target_bir_lowering=False, debug=False,
                   num_devices=NCORES)

    xt_d = nc.dram_tensor("xt", [BPC, NCH, 128, 128], bf, kind="ExternalInput")
    xtt_d = nc.dram_tensor("xtt", [BPC, TAIL, 128], bf, kind="ExternalInput")
    wp_d = nc.dram_tensor("wp", [NCH, E, 128, D], bf, kind="ExternalInput")
    wpt_d = nc.dram_tensor("wpt", [E, TAIL, D], bf, kind="ExternalInput")
    gsc_d = nc.dram_tensor("gsc", [128, BPC * nslot], f32, kind="ExternalInput")
    wix_d = nc.dram_tensor("wix", [1, BPC * nslot], i32, kind="ExternalInput")
    y_d = nc.dram_tensor("y", [BPC, 128, D], bf, kind="ExternalOutput")

    ngroups = BPC // group

    with tile.TileContext(nc) as tc:
        with (
            tc.tile_pool(name="const", bufs=1) as cpool,
            tc.tile_pool(name="xin", bufs=1) as xpool,
            tc.tile_pool(name="xs", bufs=1) as xspool,
            tc.tile_pool(name="part", bufs=1) as partpool,
            tc.tile_pool(name="outp", bufs=4) as opool,
            tc.tile_pool(name="psum", bufs=1, space="PSUM") as ppool,
        ):
            # --- tiny constants first (all input loads share the SP HWDGE
            # ring, in consumption order; stores go on the ACT ring) ---
            wix_sb = cpool.tile([1, BPC * nslot], i32, tag="wix")
            nc.scalar.dma_start(wix_sb[:], wix_d[:])
            gsc_sb = cpool.tile([128, BPC * nslot], f32, tag="gsc")
            nc.scalar.dma_start(gsc_sb[:], gsc_d[:])
            xtt_sb = cpool.tile([TAIL, BPC, 128], bf, tag="xtt")
            nc.scalar.dma_start(xtt_sb[:], xtt_d.rearrange("s p l -> p s l"))
            wt_sb = cpool.tile([TAIL, E * D], bf, tag="wt")
            nc.scalar.dma_start(wt_sb[:].rearrange("p (e n) -> p e n", e=E),
                                wpt_d.rearrange("e p n -> p e n"))

            # All expert offsets (pre-multiplied by D on the host) into PE
            # registers with a single TENSOR_LOAD.
            _, evlist = nc.values_load_multi_w_load_instructions(
                wix_sb[0:1, :],
                engines=(mybir.EngineType.PE,),
                min_val=0, max_val=(E - 1) * D,
                skip_runtime_bounds_check=True,
            )

            # --- interleave per-sample x loads into the W chunk stream so
            # each arrives just before it is consumed ---
            xt_sb = [None] * BPC

            def load_x(s):
                t = xpool.tile([128, NCH, 128], bf, tag=f"x{s}", name=f"x{s}")
                nc.sync.dma_start(t[:], xt_d[s].rearrange("c p l -> p c l"))
                xt_sb[s] = t

            w_sb = []

            def load_w(c):
                wc = cpool.tile([128, E * D], bf, tag=f"w{c}", name=f"w{c}")
                nc.sync.dma_start(wc[:].rearrange("p (e n) -> p e n", e=E),
                                  wp_d[c].rearrange("e p n -> p e n"))
                w_sb.append(wc)

            # order: x0 x1 w0 x2 x3 w1 w2 x4-7 w3 x8-11 w4 x12-15 w5 w6
            load_x(0)
            load_x(1)
            xafter = {0: range(2, 4), 2: range(4, 8), 3: range(8, 12),
                      4: range(12, 16)}
            for c in range(NCH):
                load_w(c)
                for s in xafter.get(c, ()):
                    if s < BPC:
                        load_x(s)

            # --- main loop: two K-stages (chunks 0-3, then 4-6 + tail) with
            # group rotation, so the PE has a full stint of ready matmuls as
            # soon as the first 4 W chunks have landed, and PSUM banks
            # rotate G0s0 -> G1s0 -> G0s1 -> G1s1.  Stage-0 partials spill
            # to SBUF (f32); stage 1 adds them back during the final
            # PSUM->bf16 combine. ---
            stages = [list(range(4)), list(range(4, NCH)) + ["tail"]]
            groups = [list(range(g * group, (g + 1) * group))
                      for g in range(ngroups)]

            xs_t, xst_t, partial = {}, {}, {}

            def emit_scales(ss):
                for s in ss:
                    for j in range(nslot):
                        col = s * nslot + j
                        xs = xspool.tile([128, NCH, 128], bf,
                                         tag=f"xs_{s % (2 * group)}_{j}",
                                         name=f"xs_{s}_{j}")
                        nc.vector.tensor_scalar_mul(
                            xs[:], xt_sb[s][:], gsc_sb[:, col:col + 1])
                        xst = xspool.tile([TAIL, 128], bf,
                                          tag=f"xst_{s % (2 * group)}_{j}",
                                          name=f"xst_{s}_{j}")
                        nc.vector.tensor_scalar_mul(
                            xst[:], xtt_sb[:, s, :], gsc_sb[:TAIL, col:col + 1])
                        xs_t[(s, j)] = xs
                        xst_t[(s, j)] = xst

            for ss in groups:
                emit_scales(ss)

            for stage, chunks in enumerate(stages):
                for ss in groups:
                    psums = {}
                    for s in ss:
                        psums[s] = ppool.tile([128, D], f32,
                                              tag=f"ps{s % group}",
                                              name=f"ps{stage}_{s}")
                    for ci, c in enumerate(chunks):
                        for s in ss:
                            for j in range(nslot):
                                if c == "tail":
                                    nc.tensor.matmul(
                                        out=psums[s][:],
                                        lhsT=xst_t[(s, j)][:],
                                        rhs=wt_sb[:, ds(evlist[s * nslot + j], D)],
                                        start=False,
                                        stop=(j == nslot - 1),
                                    )
                                else:
                                    nc.tensor.matmul(
                                        out=psums[s][:],
                                        lhsT=xs_t[(s, j)][:, c, :],
                                        rhs=w_sb[c][:, ds(evlist[s * nslot + j], D)],
                                        start=(ci == 0 and j == 0),
                                        stop=(stage == 0 and ci == len(chunks) - 1
                                              and j == nslot - 1),
                                    )
                    for s in ss:
                        if stage == 0:
                            part = partpool.tile([128, D], bf, tag=f"part{s}",
                                                 name=f"part{s}")
                            nc.scalar.copy(part[:], psums[s][:])
                            partial[s] = part
                        else:
                            outt = opool.tile([128, D], bf, tag="out",
                                              name=f"out{s}")
                            nc.vector.tensor_tensor(
                                out=outt[:], in0=psums[s][:],
                                in1=partial[s][:], op=mybir.AluOpType.add)
                            nc.scalar.dma_start(y_d[s], outt[:])

    nc.compile()
    return nc


def _build_program_weff(nslot, group):
    """W_eff variant: per-sample combined weights built on ACT+DVE (f32
    gates), PE does 8 static matmuls per sample.  Two halves of W
    (chunks 0-3 | chunks 4-6 + tail/bias rows) = the two K-stages."""
    import concourse.mybir as mybir
    import concourse.tile as tile
    from concourse import bacc
    from concourse.bass import ds

    bf = mybir.dt.bfloat16
    f32 = mybir.dt.float32
    i32 = mybir.dt.int32
    HW = 4 * D                     # columns per half (2048)

    nc = bacc.Bacc("TRN2", target_bir_lowering=False, debug=False,
                   num_devices=NCORES)

    xt_d = nc.dram_tensor("xt", [BPC, NCH, 128, 128], bf, kind="ExternalInput")
    xtt_d = nc.dram_tensor("xtt", [BPC, TAIL, 128], bf, kind="ExternalInput")
    wh_d = nc.dram_tensor("wh", [2, E, 128, HW], bf, kind="ExternalInput")
    gsc_d = nc.dram_tensor("gsc", [128, BPC * nslot], f32, kind="ExternalInput")
    wix_d = nc.dram_tensor("wix", [1, BPC * nslot], i32, kind="ExternalInput")
    y_d = nc.dram_tensor("y", [BPC, 128, D], bf, kind="ExternalOutput")

    ngroups = BPC // group

    with tile.TileContext(nc) as tc:
        with (
            tc.tile_pool(name="const", bufs=1) as cpool,
            tc.tile_pool(name="xin", bufs=1) as xpool,
            tc.tile_pool(name="wf", bufs=2) as wfpool,
            tc.tile_pool(name="part", bufs=1) as partpool,
            tc.tile_pool(name="outp", bufs=4) as opool,
            tc.tile_pool(name="psum", bufs=1, space="PSUM") as ppool,
        ):
            # tiny consts on the ACT ring
            wix_sb = cpool.tile([1, BPC * nslot], i32, tag="wix")
            nc.scalar.dma_start(wix_sb[:], wix_d[:])
            gsc_sb = cpool.tile([128, BPC * nslot], f32, tag="gsc")
            nc.scalar.dma_start(gsc_sb[:], gsc_d[:])
            xtt_sb = cpool.tile([TAIL, BPC, 128], bf, tag="xtt")
            nc.scalar.dma_start(xtt_sb[:], xtt_d.rearrange("s p l -> p s l"))

            # expert offsets (pre-multiplied by HW on host) for ACT + DVE
            _, evlist = nc.values_load_multi_w_load_instructions(
                wix_sb[0:1, :],
                engines=(mybir.EngineType.Activation, mybir.EngineType.DVE),
                min_val=0, max_val=(E - 1) * HW,
                skip_runtime_bounds_check=True,
            )

            xt_sb = [None] * BPC

            def load_x(s):
                t = xpool.tile([128, NCH, 128], bf, tag=f"x{s}", name=f"x{s}")
                nc.sync.dma_start(t[:], xt_d[s].rearrange("c p l -> p c l"))
                xt_sb[s] = t

            # big loads on the SP ring: x0-1, wh0, x2-7, wh1, x8-15
            load_x(0)
            load_x(1)
            wh_sb = []
            wh0 = cpool.tile([128, E * HW], bf, tag="wh0", name="wh0")
            nc.sync.dma_start(wh0[:].rearrange("p (e n) -> p e n", e=E),
                              wh_d[0].rearrange("e p n -> p e n"))
            wh_sb.append(wh0)
            for s in range(2, 8):
                load_x(s)
            wh1 = cpool.tile([128, E * HW], bf, tag="wh1", name="wh1")
            nc.sync.dma_start(wh1[:].rearrange("p (e n) -> p e n", e=E),
                              wh_d[1].rearrange("e p n -> p e n"))
            wh_sb.append(wh1)
            for s in range(8, BPC):
                load_x(s)

            groups = [list(range(g * group, (g + 1) * group))
                      for g in range(ngroups)]
            partial = {}

            for stage in range(2):
                wh = wh_sb[stage]
                for gi, ss in enumerate(groups):
                    # build W_eff for each sample of this stint
                    weff = {}
                    for s in ss:
                        cols = [s * nslot + j for j in range(nslot)]
                        # pass1 on ACT (except one sample per group on DVE
                        # to balance engine load)
                        t = wfpool.tile([128, HW], bf, tag=f"t{s % 2}",
                                        name=f"t{stage}_{s}")
                        src0 = wh[:, ds(evlist[cols[0]], HW)]
                        if s % group == group - 1:
                            nc.vector.tensor_scalar_mul(
                                t[:], src0, gsc_sb[:, cols[0]:cols[0] + 1])
                        else:
                            nc.scalar.mul(t[:], src0,
                                          gsc_sb[:, cols[0]:cols[0] + 1])
                        acc = t
                        for j in range(1, nslot):
                            dst = wfpool.tile(
                                [128, HW], bf, tag=f"wf{s % 2}_{j % 2}",
                                name=f"wf{stage}_{s}_{j}")
                            nc.vector.scalar_tensor_tensor(
                                out=dst[:],
                                in0=wh[:, ds(evlist[cols[j]], HW)],
                                scalar=gsc_sb[:, cols[j]:cols[j] + 1],
                                in1=acc[:],
                                op0=mybir.AluOpType.mult,
                                op1=mybir.AluOpType.add,
                            )
                            acc = dst
                        weff[s] = acc

                    psums = {}
                    for s in ss:
                        psums[s] = ppool.tile([128, D], f32,
                                              tag=f"ps{s % group}",
                                              name=f"ps{stage}_{s}")
                    nmain = 4 if stage == 0 else NCH - 4
                    for s in ss:
                        for ci in range(nmain):
                            nc.tensor.matmul(
                                out=psums[s][:],
                                lhsT=xt_sb[s][:, stage * 4 + ci, :],
                                rhs=weff[s][:, ci * D:(ci + 1) * D],
                                start=(ci == 0),
                                stop=(stage == 0 and ci == nmain - 1),
                            )
                        if stage == 1:
                            nc.tensor.matmul(
                                out=psums[s][:],
                                lhsT=xtt_sb[:, s, :],
                                rhs=weff[s][0:TAIL, nmain * D:(nmain + 1) * D],
                                start=False,
                                stop=True,
                            )
                    for s in ss:
                        if stage == 0:
                            part = partpool.tile([128, D], bf, tag=f"part{s}",
                                                 name=f"part{s}")
                            nc.scalar.copy(part[:], psums[s][:])
                            partial[s] = part
                        else:
                            outt = opool.tile([128, D], bf, tag="out",
                                              name=f"out{s}")
                            nc.vector.tensor_tensor(
                                out=outt[:], in0=psums[s][:],
                                in1=partial[s][:], op=mybir.AluOpType.add)
                            nc.sync.dma_start(y_d[s], outt[:])

    nc.compile()
    return nc


def _get_program(nslot):
    # 1 PSUM bank per sample; group size only bounded by SBUF for the
    # gate-scaled x copies (group * nslot tiles of ~0.23MB, double-buffered).
    group = 8 if nslot <= 2 else (4 if nslot <= 4 else 2)
    impl = os.environ.get("KERNEL_IMPL", "weff")
    key = (impl, nslot, group)
    if key not in _PROG_CACHE:
        build = _build_program_weff if impl == "weff" else _build_program
        _PROG_CACHE[key] = (build(nslot, group), group)
    return _PROG_CACHE[key]


def _prepare(x, logits, moe_masks, W, bvec, top_k):
    """Host-side glue: gating + dtype/layout prep. Returns per-core in_maps."""
    g = _gates_np(logits, moe_masks, top_k)                     # [B, E] f32
    nslot = int(top_k) if int(top_k) > 0 else E
    nslot = min(nslot, E)
    idx = np.argsort(-g, axis=1, kind="stable")[:, :nslot]      # [B, nslot]
    gv = np.take_along_axis(g, idx, axis=1).astype(np.float32)  # [B, nslot]
    idx = idx.astype(np.int32)

    xbf = x.astype(BF16)
    # [B, L, 896] -> [B, c, f, l]
    xt = np.ascontiguousarray(
        xbf[:, :, :NCH * 128].reshape(B, L, NCH, 128).transpose(0, 2, 3, 1))
    xtt = np.empty((B, TAIL, L), dtype=BF16)
    xtt[:, :TAILF, :] = xbf[:, :, NCH * 128:].transpose(0, 2, 1)
    xtt[:, TAILF, :] = BF16(1.0)                                # bias ones-row

    wbf = W.astype(BF16)
    impl = os.environ.get("KERNEL_IMPL", "weff")
    if impl == "weff":
        HWC = 4 * D
        Wm = wbf[:, :NCH * 128, :].reshape(E, NCH, 128, D)
        wh = np.zeros((2, E, 128, HWC), dtype=BF16)
        wh[0] = Wm[:, 0:4].transpose(0, 2, 1, 3).reshape(E, 128, 4 * D)
        wh[1, :, :, :3 * D] = \
            Wm[:, 4:7].transpose(0, 2, 1, 3).reshape(E, 128, 3 * D)
        wh[1, :, :TAILF, 3 * D:] = wbf[:, NCH * 128:, :].transpose(0, 1, 2)
        wh[1, :, TAILF, 3 * D:] = bvec.astype(BF16)             # bias row
        wmul = HWC
    else:
        wp = np.ascontiguousarray(
            wbf[:, :NCH * 128, :].reshape(E, NCH, 128, D).transpose(1, 0, 2, 3))
        wpt = np.empty((E, TAIL, D), dtype=BF16)
        wpt[:, :TAILF, :] = wbf[:, NCH * 128:, :]
        wpt[:, TAILF, :] = bvec.astype(BF16)                    # bias row
        wmul = D

    in_maps = []
    for k in range(NCORES):
        s0, s1 = k * BPC, (k + 1) * BPC
        gslice = gv[s0:s1].reshape(1, BPC * nslot)
        m = {
            "xt": np.ascontiguousarray(xt[s0:s1]),
            "xtt": np.ascontiguousarray(xtt[s0:s1]),
            "gsc": np.ascontiguousarray(
                np.broadcast_to(gslice, (128, BPC * nslot))).astype(np.float32),
            "wix": (idx[s0:s1].reshape(1, BPC * nslot) * wmul).astype(np.int32),
        }
        if impl == "weff":
            m["wh"] = wh
        else:
            m["wp"] = wp
            m["wpt"] = wpt
        in_maps.append(m)
    return in_maps, nslot


def kernel(cycle_curve_data, logits, moe_masks, W, b, top_k):
    global LAST_RESULT
    from concourse import bass_utils

    x = np.asarray(cycle_curve_data, dtype=np.float32)
    logits = np.asarray(logits, dtype=np.float32)
    moe_masks = np.asarray(moe_masks)
    W = np.asarray(W, dtype=np.float32)
    bvec = np.asarray(b, dtype=np.float32)
    top_k = int(np.asarray(top_k))

    in_maps, nslot = _prepare(x, logits, moe_masks, W, bvec, top_k)
    nc, _group = _get_program(nslot)

    res = bass_utils.run_bass_kernel_spmd(
        nc, in_maps, core_ids=list(range(NCORES)),
        trace=bool(int(os.environ.get("KERNEL_TRACE", "0"))),
    )
    LAST_RESULT = res

    out = np.concatenate([np.asarray(r["y"]) for r in res.results], axis=0)
    return out.astype(BF16), np.float32(0.0), np.float32(0.0)


# revision 21
# speedup vs baseline: 1.4159x; 1.4159x over previous
"""Trainium2 Bass kernel for nn_BatteryMoEFlattenIntraCycleMoELayer.

Computes, for B=128 battery samples of shape [L=128, F=900]:
    g = renorm(top_k(softmax(logits) * mask))          # [B, E=8] gates
    out[b] = sum_e g[b,e] * (x[b] @ W[e] + b[e])       # [B, L, D=512], bf16

Strategy: data-parallel over B across 8 cores (16 samples/core), W replicated.
Host does gating (tiny [128,8] math) + layout/dtype glue only; all O(B*L*F*D)
FLOPs run on device.  Per sample the device computes the two selected experts'
matmuls in bf16 (x pre-scaled by the gate), accumulating both experts and the
gated bias (via a ones-row folded into the K dimension) into one PSUM bank.
Expert selection is a runtime register read (values_load) driving a dynamic
access pattern into SBUF-resident W.
"""

import os

import numpy as np
import ml_dtypes

B, L, FIN, D, E = 128, 128, 900, 512, 8
NCORES = 8
BPC = B // NCORES            # samples per core
NCH = FIN // 128             # 7 full 128-row contraction chunks
TAILF = FIN - NCH * 128      # 4 leftover features
TAIL = TAILF + 1             # + ones row (folds bias into the matmul)
EPS = 1e-9

BF16 = ml_dtypes.bfloat16

LAST_RESULT = None           # BassKernelResults of the most recent run
_PROG_CACHE = {}


def _gates_np(logits, moe_masks, top_k):
    """Numpy replica of reference._gates (f32, same tie-breaking as lax.top_k)."""
    logits = logits.astype(np.float32)
    mask = (moe_masks == 1).astype(np.float32)
    x = logits - logits.max(axis=1, keepdims=True)
    ex = np.exp(x)
    sm = ex / ex.sum(axis=1, keepdims=True)
    g = sm * mask
    if top_k > 0:
        idx = np.argsort(-g, axis=1, kind="stable")[:, :top_k]
        tk = np.zeros_like(g)
        np.put_along_axis(tk, idx, np.float32(1.0), axis=1)
        g = g * tk
    return g / (g.sum(axis=1, keepdims=True) + np.float32(EPS))


def _build_program(nslot, group):
    """Build the (SPMD, per-core) Bass program: BPC samples, nslot expert
    slots per sample, `group` samples sharing a round of PSUM banks."""
    import concourse.mybir as mybir
    import concourse.tile as tile
    from concourse import bacc
    from concourse.bass import ds

    bf = mybir.dt.bfloat16
    f32 = mybir.dt.float32
    i32 = mybir.dt.int32

    nc = bacc.Bacc("TRN2", target_bir_lowering=False, debug=False,
                   num_devices=NCORES)

    K2 = nslot * TAIL            # stacked tail rows (both experts + bias)
    xt_d = nc.dram_tensor("xt", [BPC, NCH, 128, 128], bf, kind="ExternalInput")
    xtt_d = nc.dram_tensor("xtt", [BPC, K2, 128], bf, kind="ExternalInput")
    wp_d = nc.dram_tensor("wp", [NCH, E, 128, D], bf, kind="ExternalInput")
    wtp_d = nc.dram_tensor("wtp", [BPC, K2, D], bf, kind="ExternalInput")
    gsc_d = nc.dram_tensor("gsc", [128, BPC * nslot], f32, kind="ExternalInput")
    gt2_d = nc.dram_tensor("gt2", [BPC, K2], f32, kind="ExternalInput")
    wix_d = nc.dram_tensor("wix", [1, BPC * nslot], i32, kind="ExternalInput")
    y_d = nc.dram_tensor("y", [BPC, 128, D], bf, kind="ExternalOutput")

    ngroups = BPC // group

    with tile.TileContext(nc) as tc:
        with (
            tc.tile_pool(name="const", bufs=1) as cpool,
            tc.tile_pool(name="xin", bufs=1) as xpool,
            tc.tile_pool(name="xs", bufs=1) as xspool,
            tc.tile_pool(name="part", bufs=1) as partpool,
            tc.tile_pool(name="outp", bufs=4) as opool,
            tc.tile_pool(name="psum", bufs=1, space="PSUM") as ppool,
        ):
            # --- tiny constants first (all input loads share the SP HWDGE
            # ring, in consumption order; stores go on the ACT ring) ---
            wix_sb = cpool.tile([1, BPC * nslot], i32, tag="wix")
            nc.scalar.dma_start(wix_sb[:], wix_d[:])
            gsc_sb = cpool.tile([128, BPC * nslot], f32, tag="gsc")
            nc.scalar.dma_start(gsc_sb[:], gsc_d[:])
            xtt_sb = cpool.tile([K2, BPC, 128], bf, tag="xtt")
            nc.scalar.dma_start(xtt_sb[:], xtt_d.rearrange("s p l -> p s l"))
            wtp_sb = cpool.tile([K2, BPC, D], bf, tag="wtp")
            nc.scalar.dma_start(wtp_sb[:], wtp_d.rearrange("s p n -> p s n"))
            gt2_sb = cpool.tile([K2, BPC], f32, tag="gt2")
            nc.scalar.dma_start(gt2_sb[:], gt2_d.rearrange("s p -> p s"))

            # All expert offsets (pre-multiplied by D on the host) into PE
            # registers with a single TENSOR_LOAD.
            _, evlist = nc.values_load_multi_w_load_instructions(
                wix_sb[0:1, :],
                engines=(mybir.EngineType.PE,),
                min_val=0, max_val=(E - 1) * D,
                skip_runtime_bounds_check=True,
            )

            # --- interleave per-sample x loads into the W chunk stream so
            # each arrives just before it is consumed ---
            xt_sb = [None] * BPC

            def load_x(s):
                t = xpool.tile([128, NCH, 128], bf, tag=f"x{s}", name=f"x{s}")
                nc.sync.dma_start(t[:], xt_d[s].rearrange("c p l -> p c l"))
                xt_sb[s] = t

            w_sb = []

            def load_w(c):
                wc = cpool.tile([128, E * D], bf, tag=f"w{c}", name=f"w{c}")
                nc.sync.dma_start(wc[:].rearrange("p (e n) -> p e n", e=E),
                                  wp_d[c].rearrange("e p n -> p e n"))
                w_sb.append(wc)

            # order: x0 x1 w0 x2 x3 w1 w2 x4-7 w3 x8-11 w4 x12-15 w5 w6
            load_x(0)
            load_x(1)
            xafter = {0: range(2, 4), 2: range(4, 8), 3: range(8, 12),
                      4: range(12, 16)}
            for c in range(NCH):
                load_w(c)
                for s in xafter.get(c, ()):
                    if s < BPC:
                        load_x(s)

            # --- main loop: two K-stages (chunks 0-3, then 4-6 + tail) with
            # group rotation, so the PE has a full stint of ready matmuls as
            # soon as the first 4 W chunks have landed, and PSUM banks
            # rotate G0s0 -> G1s0 -> G0s1 -> G1s1.  Stage-0 partials spill
            # to SBUF (f32); stage 1 adds them back during the final
            # PSUM->bf16 combine. ---
            stages = [list(range(4)), list(range(4, NCH)) + ["tail"]]
            groups = [list(range(g * group, (g + 1) * group))
                      for g in range(ngroups)]

            xs_t, xst_t, partial = {}, {}, {}

            def emit_scales(ss):
                for s in ss:
                    for j in range(nslot):
                        col = s * nslot + j
                        xs = xspool.tile([128, NCH, 128], bf,
                                         tag=f"xs_{s % (2 * group)}_{j}",
                                         name=f"xs_{s}_{j}")
                        nc.vector.tensor_scalar_mul(
                            xs[:], xt_sb[s][:], gsc_sb[:, col:col + 1])
                        xs_t[(s, j)] = xs
                    xstp = xspool.tile([K2, 128], bf,
                                       tag=f"xst_{s % (2 * group)}",
                                       name=f"xst_{s}")
                    nc.vector.tensor_scalar_mul(
                        xstp[:], xtt_sb[:, s, :], gt2_sb[:, s:s + 1])
                    xst_t[s] = xstp

            for ss in groups:
                emit_scales(ss)

            for stage, chunks in enumerate(stages):
                for ss in groups:
                    psums = {}
                    for s in ss:
                        psums[s] = ppool.tile([128, D], f32,
                                              tag=f"ps{s % group}",
                                              name=f"ps{stage}_{s}")
                    for ci, c in enumerate(chunks):
                        for s in ss:
                            if c == "tail":
                                nc.tensor.matmul(
                                    out=psums[s][:],
                                    lhsT=xst_t[s][:],
                                    rhs=wtp_sb[:, s, :],
                                    start=False,
                                    stop=True,
                                )
                                continue
                            for j in range(nslot):
                                nc.tensor.matmul(
                                    out=psums[s][:],
                                    lhsT=xs_t[(s, j)][:, c, :],
                                    rhs=w_sb[c][:, ds(evlist[s * nslot + j], D)],
                                    start=(ci == 0 and j == 0),
                                    stop=(stage == 0 and ci == len(chunks) - 1
                                          and j == nslot - 1),
                                )
                    for s in ss:
                        if stage == 0:
                            part = partpool.tile([128, D], bf, tag=f"part{s}",
                                                 name=f"part{s}")
                            nc.scalar.copy(part[:], psums[s][:])
                            partial[s] = part
                        else:
                            outt = opool.tile([128, D], bf, tag="out",
                                              name=f"out{s}")
                            nc.vector.tensor_tensor(
                                out=outt[:], in0=psums[s][:],
                                in1=partial[s][:], op=mybir.AluOpType.add)
                            nc.scalar.dma_start(y_d[s], outt[:])

    nc.compile()
    return nc


def _build_program_weff(nslot, group):
    """W_eff variant: per-sample combined weights built on ACT+DVE (f32
    gates), PE does 8 static matmuls per sample.  Two halves of W
    (chunks 0-3 | chunks 4-6 + tail/bias rows) = the two K-stages."""
    import concourse.mybir as mybir
    import concourse.tile as tile
    from concourse import bacc
    from concourse.bass import ds

    bf = mybir.dt.bfloat16
    f32 = mybir.dt.float32
    i32 = mybir.dt.int32
    HW = 4 * D                     # columns per half (2048)

    nc = bacc.Bacc("TRN2", target_bir_lowering=False, debug=False,
                   num_devices=NCORES)

    xt_d = nc.dram_tensor("xt", [BPC, NCH, 128, 128], bf, kind="ExternalInput")
    xtt_d = nc.dram_tensor("xtt", [BPC, TAIL, 128], bf, kind="ExternalInput")
    wh_d = nc.dram_tensor("wh", [2, E, 128, HW], bf, kind="ExternalInput")
    gsc_d = nc.dram_tensor("gsc", [128, BPC * nslot], f32, kind="ExternalInput")
    wix_d = nc.dram_tensor("wix", [1, BPC * nslot], i32, kind="ExternalInput")
    y_d = nc.dram_tensor("y", [BPC, 128, D], bf, kind="ExternalOutput")

    ngroups = BPC // group

    with tile.TileContext(nc) as tc:
        with (
            tc.tile_pool(name="const", bufs=1) as cpool,
            tc.tile_pool(name="xin", bufs=1) as xpool,
            tc.tile_pool(name="wf", bufs=2) as wfpool,
            tc.tile_pool(name="part", bufs=1) as partpool,
            tc.tile_pool(name="outp", bufs=4) as opool,
            tc.tile_pool(name="psum", bufs=1, space="PSUM") as ppool,
        ):
            # tiny consts on the ACT ring
            wix_sb = cpool.tile([1, BPC * nslot], i32, tag="wix")
            nc.scalar.dma_start(wix_sb[:], wix_d[:])
            gsc_sb = cpool.tile([128, BPC * nslot], f32, tag="gsc")
            nc.scalar.dma_start(gsc_sb[:], gsc_d[:])
            xtt_sb = cpool.tile([TAIL, BPC, 128], bf, tag="xtt")
            nc.scalar.dma_start(xtt_sb[:], xtt_d.rearrange("s p l -> p s l"))

            # expert offsets (pre-multiplied by HW on host) for ACT + DVE
            _, evlist = nc.values_load_multi_w_load_instructions(
                wix_sb[0:1, :],
                engines=(mybir.EngineType.Activation, mybir.EngineType.DVE),
                min_val=0, max_val=(E - 1) * HW,
                skip_runtime_bounds_check=True,
            )

            xt_sb = [None] * BPC

            def load_x(s):
                t = xpool.tile([128, NCH, 128], bf, tag=f"x{s}", name=f"x{s}")
                nc.sync.dma_start(t[:], xt_d[s].rearrange("c p l -> p c l"))
                xt_sb[s] = t

            # big loads on the SP ring: x0-1, wh0, x2-7, wh1, x8-15
            load_x(0)
            load_x(1)
            wh_sb = []
            wh0 = cpool.tile([128, E * HW], bf, tag="wh0", name="wh0")
            nc.sync.dma_start(wh0[:].rearrange("p (e n) -> p e n", e=E),
                              wh_d[0].rearrange("e p n -> p e n"))
            wh_sb.append(wh0)
            for s in range(2, 8):
                load_x(s)
            wh1 = cpool.tile([128, E * HW], bf, tag="wh1", name="wh1")
            nc.sync.dma_start(wh1[:].rearrange("p (e n) -> p e n", e=E),
                              wh_d[1].rearrange("e p n -> p e n"))
            wh_sb.append(wh1)
            for s in range(8, BPC):
                load_x(s)

            groups = [list(range(g * group, (g + 1) * group))
                      for g in range(ngroups)]
            partial = {}

            for stage in range(2):
                wh = wh_sb[stage]
                for gi, ss in enumerate(groups):
                    # build W_eff for each sample of this stint
                    weff = {}
                    for s in ss:
                        cols = [s * nslot + j for j in range(nslot)]
                        # pass1 on ACT (except one sample per group on DVE
                        # to balance engine load)
                        t = wfpool.tile([128, HW], bf, tag=f"t{s % 2}",
                                        name=f"t{stage}_{s}")
                        src0 = wh[:, ds(evlist[cols[0]], HW)]
                        if s % group == group - 1:
                            nc.vector.tensor_scalar_mul(
                                t[:], src0, gsc_sb[:, cols[0]:cols[0] + 1])
                        else:
                            nc.scalar.mul(t[:], src0,
                                          gsc_sb[:, cols[0]:cols[0] + 1])
                        acc = t
                        for j in range(1, nslot):
                            dst = wfpool.tile(
                                [128, HW], bf, tag=f"wf{s % 2}_{j % 2}",
                                name=f"wf{stage}_{s}_{j}")
                            nc.vector.scalar_tensor_tensor(
                                out=dst[:],
                                in0=wh[:, ds(evlist[cols[j]], HW)],
                                scalar=gsc_sb[:, cols[j]:cols[j] + 1],
                                in1=acc[:],
                                op0=mybir.AluOpType.mult,
                                op1=mybir.AluOpType.add,
                            )
                            acc = dst
                        weff[s] = acc

                    psums = {}
                    for s in ss:
                        psums[s] = ppool.tile([128, D], f32,
                                              tag=f"ps{s % group}",
                                              name=f"ps{stage}_{s}")
                    nmain = 4 if stage == 0 else NCH - 4
                    for s in ss:
                        for ci in range(nmain):
                            nc.tensor.matmul(
                                out=psums[s][:],
                                lhsT=xt_sb[s][:, stage * 4 + ci, :],
                                rhs=weff[s][:, ci * D:(ci + 1) * D],
                                start=(ci == 0),
                                stop=(stage == 0 and ci == nmain - 1),
                            )
                        if stage == 1:
                            nc.tensor.matmul(
                                out=psums[s][:],
                                lhsT=xtt_sb[:, s, :],
                                rhs=weff[s][0:TAIL, nmain * D:(nmain + 1) * D],
                                start=False,
                                stop=True,
                            )
                    for s in ss:
                        if stage == 0:
                            part = partpool.tile([128, D], bf, tag=f"part{s}",
                                                 name=f"part{s}")
                            nc.scalar.copy(part[:], psums[s][:])
                            partial[s] = part
                        else:
                            outt = opool.tile([128, D], bf, tag="out",
                                              name=f"out{s}")
                            nc.vector.tensor_tensor(
                                out=outt[:], in0=psums[s][:],
                                in1=partial[s][:], op=mybir.AluOpType.add)
                            nc.sync.dma_start(y_d[s], outt[:])

    nc.compile()
    return nc


def _get_program(nslot):
    # 1 PSUM bank per sample; group size only bounded by SBUF for the
    # gate-scaled x copies (group * nslot tiles of ~0.23MB, double-buffered).
    group = 8 if nslot <= 2 else (4 if nslot <= 4 else 2)
    impl = os.environ.get("KERNEL_IMPL", "dual")
    key = (impl, nslot, group)
    if key not in _PROG_CACHE:
        build = _build_program_weff if impl == "weff" else _build_program
        _PROG_CACHE[key] = (build(nslot, group), group)
    return _PROG_CACHE[key]


def _prepare(x, logits, moe_masks, W, bvec, top_k):
    """Host-side glue: gating + dtype/layout prep. Returns per-core in_maps."""
    g = _gates_np(logits, moe_masks, top_k)                     # [B, E] f32
    nslot = int(top_k) if int(top_k) > 0 else E
    nslot = min(nslot, E)
    idx = np.argsort(-g, axis=1, kind="stable")[:, :nslot]      # [B, nslot]
    gv = np.take_along_axis(g, idx, axis=1).astype(np.float32)  # [B, nslot]
    idx = idx.astype(np.int32)

    xbf = x.astype(BF16)
    # [B, L, 896] -> [B, c, f, l]
    xt = np.ascontiguousarray(
        xbf[:, :, :NCH * 128].reshape(B, L, NCH, 128).transpose(0, 2, 3, 1))
    xtt1 = np.empty((B, TAIL, L), dtype=BF16)
    xtt1[:, :TAILF, :] = xbf[:, :, NCH * 128:].transpose(0, 2, 1)
    xtt1[:, TAILF, :] = BF16(1.0)                               # bias ones-row

    wbf = W.astype(BF16)
    impl = os.environ.get("KERNEL_IMPL", "dual")
    if impl == "weff":
        HWC = 4 * D
        Wm = wbf[:, :NCH * 128, :].reshape(E, NCH, 128, D)
        wh = np.zeros((2, E, 128, HWC), dtype=BF16)
        wh[0] = Wm[:, 0:4].transpose(0, 2, 1, 3).reshape(E, 128, 4 * D)
        wh[1, :, :, :3 * D] = \
            Wm[:, 4:7].transpose(0, 2, 1, 3).reshape(E, 128, 3 * D)
        wh[1, :, :TAILF, 3 * D:] = wbf[:, NCH * 128:, :].transpose(0, 1, 2)
        wh[1, :, TAILF, 3 * D:] = bvec.astype(BF16)             # bias row
        wmul = HWC
    else:
        wp = np.ascontiguousarray(
            wbf[:, :NCH * 128, :].reshape(E, NCH, 128, D).transpose(1, 0, 2, 3))
        wpt = np.empty((E, TAIL, D), dtype=BF16)
        wpt[:, :TAILF, :] = wbf[:, NCH * 128:, :]
        wpt[:, TAILF, :] = bvec.astype(BF16)                    # bias row
        # per-sample stacked tail: rows j*TAIL..(j+1)*TAIL = expert e_j's
        # tail features + bias row; matching x-tail replicated per slot and
        # a per-row gate vector so one DVE op scales the whole stack
        K2 = nslot * TAIL
        wtp = wpt[idx.reshape(-1)].reshape(B, K2, D)            # [B, K2, D]
        xtt = np.ascontiguousarray(
            np.repeat(xtt1[:, None, :, :], nslot, axis=1).reshape(B, K2, L))
        gt2 = np.repeat(gv, TAIL, axis=1).astype(np.float32)    # [B, K2]
        wmul = D

    in_maps = []
    for k in range(NCORES):
        s0, s1 = k * BPC, (k + 1) * BPC
        gslice = gv[s0:s1].reshape(1, BPC * nslot)
        m = {
            "xt": np.ascontiguousarray(xt[s0:s1]),
            "gsc": np.ascontiguousarray(
                np.broadcast_to(gslice, (128, BPC * nslot))).astype(np.float32),
            "wix": (idx[s0:s1].reshape(1, BPC * nslot) * wmul).astype(np.int32),
        }
        if impl == "weff":
            m["wh"] = wh
            m["xtt"] = np.ascontiguousarray(xtt1[s0:s1])
        else:
            m["wp"] = wp
            m["xtt"] = np.ascontiguousarray(xtt[s0:s1])
            m["wtp"] = np.ascontiguousarray(wtp[s0:s1])
            m["gt2"] = np.ascontiguousarray(gt2[s0:s1])
        in_maps.append(m)
    return in_maps, nslot


def kernel(cycle_curve_data, logits, moe_masks, W, b, top_k):
    global LAST_RESULT
    from concourse import bass_utils

    x = np.asarray(cycle_curve_data, dtype=np.float32)
    logits = np.asarray(logits, dtype=np.float32)
    moe_masks = np.asarray(moe_masks)
    W = np.asarray(W, dtype=np.float32)
    bvec = np.asarray(b, dtype=np.float32)
    top_k = int(np.asarray(top_k))

    in_maps, nslot = _prepare(x, logits, moe_masks, W, bvec, top_k)
    nc, _group = _get_program(nslot)

    res = bass_utils.run_bass_kernel_spmd(
        nc, in_maps, core_ids=list(range(NCORES)),
        trace=bool(int(os.environ.get("KERNEL_TRACE", "0"))),
    )
    LAST_RESULT = res

    out = np.concatenate([np.asarray(r["y"]) for r in res.results], axis=0)
    return out.astype(BF16), np.float32(0.0), np.float32(0.0)
